# revision 1
# baseline (speedup 1.0000x reference)
"""Trainium2 Bass kernel: GroupNorm + single-head self-attention + residual.

Reference computation (B=4, C=512, H=W=64, N=4096 tokens):
    h  = GroupNorm32(x) ; hf = h tokens x channels
    q/k/v = hf @ W{q,k,v}^T + b
    attn  = softmax(q k^T / sqrt(C)) @ v
    out   = attn @ Wo^T + bo  (+ x residual)

Sharding: 8 cores, core c -> batch b=c//2, query-half h=c%2 (2048 queries).
Each core receives x[b] with tokens rotated so its query half is first; the
SPMD graph is identical on every core. K/V are computed for all 4096 tokens
on both cores of a pair (cheaper than a collective at this size).

On-chip layouts (partition dim first):
    xn  [128, 4, 4096] bf16   normalized input, channel c = ci*128+p
    kt  [128, 4, 4096] bf16   K^T, d on partitions
    qt  [128, 4, 2048] bf16   Q^T * (1/sqrt(C)), d on partitions
    v   [128, 32, 512] bf16   V, tokens on partitions
Scores are built transposed (S'[s,t] = sum_d kt*qt) so that softmax
normalization is a column sum (ones-matmul over partitions) and the
attention matmul attn^T[c,t] = sum_s v[s,c] P'[s,t] needs no transposes.
Softmax is max-free (scores are ~N(0,1); exp cannot overflow fp32).
"""

import math
import os

import numpy as np
import ml_dtypes

import concourse.bass as bass
import concourse.bacc as bacc
import concourse.mybir as mybir
import concourse.tile as tile
from concourse.bass_utils import run_bass_kernel_spmd

# ----------------------------------------------------------------------------
# Problem constants (hardcoded per spec: x [4, 512, 64, 64] f32)
B, C, H, W = 4, 512, 64, 64
N = H * W          # 4096 tokens
T = N // 2         # 2048 queries per core
P = 128
CT = C // P        # 4 channel tiles
NUM_GROUPS = 32
GSIZE = C // NUM_GROUPS  # 16 channels per group
EPS = 1e-5
SCL = 1.0 / math.sqrt(C)
N_CORES = 8
F32 = mybir.dt.float32
BF16 = mybir.dt.bfloat16

_AF = mybir.ActivationFunctionType
_ALU = mybir.AluOpType

# set by kernel() when BASS_KERNEL_TRACE=1 (used by test.py)
last_exec_time_ns = None
last_results = None


def _build_graph():
    from contextlib import ExitStack

    # Bacc (not plain Bass): its compile() runs generate_event_semaphores,
    # which splits multi-wait sync_info into InstEventSemaphores — this
    # walrus build rejects >2 waits per instruction.
    nc = bacc.Bacc("TRN2", target_bir_lowering=False)

    x_ext = nc.declare_dram_parameter("x", [C, N], BF16, isOutput=False)
    wqt_ext = nc.declare_dram_parameter("wqt", [P, CT, C], BF16, isOutput=False)
    wkt_ext = nc.declare_dram_parameter("wkt", [P, CT, C], BF16, isOutput=False)
    wvt_ext = nc.declare_dram_parameter("wvt", [P, CT, C], BF16, isOutput=False)
    wot_ext = nc.declare_dram_parameter("wot", [P, CT, C], BF16, isOutput=False)
    bqs_ext = nc.declare_dram_parameter("bqs", [P, CT], F32, isOutput=False)
    bkp_ext = nc.declare_dram_parameter("bkp", [P, CT], F32, isOutput=False)
    bop_ext = nc.declare_dram_parameter("bop", [P, CT], F32, isOutput=False)
    bvr_ext = nc.declare_dram_parameter("bvrep", [P, C], F32, isOutput=False)
    gsc_ext = nc.declare_dram_parameter("gnsc", [P, CT], F32, isOutput=False)
    gbi_ext = nc.declare_dram_parameter("gnbi", [P, CT], F32, isOutput=False)
    gind_ext = nc.declare_dram_parameter("gind", [P, CT, NUM_GROUPS], F32, isOutput=False)
    gindt_ext = nc.declare_dram_parameter("gindt", [NUM_GROUPS, CT, P], F32, isOutput=False)
    onesq_ext = nc.declare_dram_parameter("ones_sq", [P, P], F32, isOutput=False)
    onesqb_ext = nc.declare_dram_parameter("ones_sq_bf", [P, P], BF16, isOutput=False)
    out_ext = nc.declare_dram_parameter("out", [C, T], F32, isOutput=True)

    SCH = N // P     # 32 s-chunks of 128
    NK = N // 512    # 8 s-chunks of 512
    TCH = T // 512   # 4 t-chunks of 512

    with tile.TileContext(nc) as tc, ExitStack() as ctx:
        consts = ctx.enter_context(tc.tile_pool(name="consts", bufs=1))
        big = ctx.enter_context(tc.tile_pool(name="big", bufs=1))
        small = ctx.enter_context(tc.tile_pool(name="small", bufs=1))

        # ---- constants into SBUF
        # weights/consts on the Activation HWDGE queue so the x load has the
        # SP queue to itself
        wqt = consts.tile([P, CT, C], BF16, tag="wqt")
        wkt = consts.tile([P, CT, C], BF16, tag="wkt")
        wvt = consts.tile([P, CT, C], BF16, tag="wvt")
        wot = consts.tile([P, CT, C], BF16, tag="wot")
        bqs = consts.tile([P, CT], F32, tag="bqs")
        bkp = consts.tile([P, CT], F32, tag="bkp")
        bop = consts.tile([P, CT], F32, tag="bop")
        bvr = consts.tile([P, C], F32, tag="bvr")
        gsc = consts.tile([P, CT], F32, tag="gsc")
        gbi = consts.tile([P, CT], F32, tag="gbi")
        gind = consts.tile([P, CT, NUM_GROUPS], F32, tag="gind")
        gindt = consts.tile([NUM_GROUPS, CT, P], F32, tag="gindt")
        onesq = consts.tile([P, P], F32, tag="onesq")
        onesqb = consts.tile([P, P], BF16, tag="onesqb")

        # ---- persistent big tensors
        xn = big.tile([P, CT, N], BF16, tag="xn")
        kt = big.tile([P, CT, N], BF16, tag="kt")
        vt = big.tile([P, SCH, C], BF16, tag="vt")
        qt = big.tile([P, CT, T], BF16, tag="qt")

        # ---- x loads first (x is the critical path): each tile split into
        # column halves across the two HWDGE queues so tile ti completes at
        # ~(ti+1) * 3.3us instead of serialized full tiles
        Nh = N // 2
        for ti in range(CT):
            nc.sync.dma_start(xn[:, ti, 0:Nh], x_ext[ti * P:(ti + 1) * P, 0:Nh])
            nc.scalar.dma_start(xn[:, ti, Nh:N], x_ext[ti * P:(ti + 1) * P, Nh:N])
        # All consts on the sync queue (small first, then weights) so the
        # scalar queue carries nothing but x halves — otherwise the ACT
        # engine blocks on a full DMA ring and the stat squares start late.
        nc.sync.dma_start(gind[:], gind_ext[:])
        nc.sync.dma_start(gindt[:], gindt_ext[:])
        nc.sync.dma_start(gsc[:], gsc_ext[:])
        nc.sync.dma_start(gbi[:], gbi_ext[:])
        nc.sync.dma_start(bqs[:], bqs_ext[:])
        nc.sync.dma_start(bkp[:], bkp_ext[:])
        nc.sync.dma_start(bop[:], bop_ext[:])
        nc.sync.dma_start(onesq[:], onesq_ext[:])
        nc.sync.dma_start(onesqb[:], onesqb_ext[:])
        nc.sync.dma_start(bvr[:], bvr_ext[:])
        nc.sync.dma_start(wkt[:], wkt_ext[:])
        nc.sync.dma_start(wqt[:], wqt_ext[:])
        nc.sync.dma_start(wvt[:], wvt_ext[:])
        nc.sync.dma_start(wot[:], wot_ext[:])

        # ---- phase 1: stats, normalize in place.  Sums and sums-of-squares
        # live in separate tiles so the DVE and ACT stat passes don't pick up
        # a false same-tile WAW serialization.  Stats run per COLUMN HALF so
        # each starts as soon as that half's DMA lands.
        sums_sb = small.tile([P, 2 * CT], F32, tag="sums_sb")
        sq_sb = small.tile([P, 2 * CT], F32, tag="sq_sb")
        with (
            tc.tile_pool(name="ph1ps", bufs=2, space="PSUM") as ph1ps,
            tc.tile_pool(name="ph1sb", bufs=2) as ph1sb,
            tc.tile_pool(name="sqpool", bufs=2) as sqpool,
        ):
            NCOL = 2 * CT
            for ti in range(CT):
                for hf in range(2):
                    col = 2 * ti + hf
                    xsl = xn[:, ti, hf * Nh:(hf + 1) * Nh]
                    # squares on ACT via accum_out; sums on DVE
                    sq = sqpool.tile([P, Nh], BF16, tag="sq", name=f"sq{col}")
                    nc.scalar.activation(sq[:], xsl, _AF.Square,
                                         accum_out=sq_sb[:, col:col + 1])
                    nc.vector.reduce_sum(sums_sb[:, col:col + 1], xsl,
                                         axis=mybir.AxisListType.X)

            # group stats via indicator matmuls, one PSUM accumulator per kind
            sums_ps = ph1ps.tile([NUM_GROUPS, 1], F32, tag="statps")
            sq_ps = ph1ps.tile([NUM_GROUPS, 1], F32, tag="statps2")
            for col in range(NCOL):
                nc.tensor.matmul(sums_ps[:], gind[:, col // 2, :],
                                 sums_sb[:, col:col + 1],
                                 start=(col == 0), stop=(col == NCOL - 1))
            for col in range(NCOL):
                nc.tensor.matmul(sq_ps[:], gind[:, col // 2, :],
                                 sq_sb[:, col:col + 1],
                                 start=(col == 0), stop=(col == NCOL - 1))
            gstats = ph1sb.tile([NUM_GROUPS, 2], F32, tag="gstats")
            inv_cnt = 1.0 / float(GSIZE * N)
            nc.vector.tensor_scalar_mul(gstats[:, 0:1], sums_ps[:], inv_cnt)
            nc.vector.tensor_scalar_mul(gstats[:, 1:2], sq_ps[:], inv_cnt)
            m2 = ph1sb.tile([NUM_GROUPS, 1], F32, tag="m2")
            nc.vector.tensor_mul(out=m2[:], in0=gstats[:, 0:1], in1=gstats[:, 0:1])
            var = ph1sb.tile([NUM_GROUPS, 1], F32, tag="var")
            nc.vector.tensor_tensor(var[:], gstats[:, 1:2], m2[:], _ALU.subtract)
            eps_t = ph1sb.tile([NUM_GROUPS, 1], F32, tag="eps")
            nc.vector.memset(eps_t[:], EPS)
            std = ph1sb.tile([NUM_GROUPS, 1], F32, tag="std")
            nc.scalar.activation(std[:], var[:], _AF.Sqrt, bias=eps_t[:])
            gmr = ph1sb.tile([NUM_GROUPS, 2], F32, tag="gmr")
            nc.vector.tensor_copy(out=gmr[:, 0:1], in_=gstats[:, 0:1])
            nc.vector.reciprocal(gmr[:, 1:2], std[:])

            # broadcast (mean, rstd) back to channel layout; A/B affine.
            # Gather all four tiles' (mean, rstd) into one [P, CT, 2] tile so
            # A and B come out of three wide DVE ops instead of a long chain.
            A_sb = small.tile([P, CT], F32, tag="A_sb")
            B_sb = small.tile([P, CT], F32, tag="B_sb")
            chan_all = ph1sb.tile([P, CT, 2], F32, tag="chanall")
            for ti in range(CT):
                chan_ps = ph1ps.tile([P, 2], F32, tag="chanps")
                nc.tensor.matmul(chan_ps[:], gindt[:, ti, :], gmr[:],
                                 start=True, stop=True)
                nc.vector.tensor_copy(out=chan_all[:, ti, :], in_=chan_ps[:])
            nc.vector.tensor_mul(out=A_sb[:], in0=chan_all[:, :, 1], in1=gsc[:])
            tmpm = ph1sb.tile([P, CT], F32, tag="tmpm")
            nc.vector.tensor_mul(out=tmpm[:], in0=chan_all[:, :, 0], in1=A_sb[:])
            nc.vector.tensor_tensor(B_sb[:], gbi[:], tmpm[:], _ALU.subtract)

            # normalize in place, column-chunk first so projections can start
            # on chunk 0 while later chunks normalize; DVE is ~2x faster than
            # ACT here so it takes 2 of every 3 slices
            for cc in range(N // 512):
                for ti in range(CT):
                    sl = xn[:, ti, cc * 512:(cc + 1) * 512]
                    if (cc * CT + ti) % 3 == 0:
                        nc.scalar.activation(sl, sl, _AF.Identity,
                                             scale=A_sb[:, ti:ti + 1],
                                             bias=B_sb[:, ti:ti + 1])
                    else:
                        nc.vector.tensor_scalar(sl, sl,
                                                A_sb[:, ti:ti + 1],
                                                B_sb[:, ti:ti + 1],
                                                _ALU.mult, _ALU.add)

        # ---- phase 2: projections, ordered by xn column chunk so they chase
        # the normalize pass.  An extra phase-2-only PSUM pool widens the
        # in-flight chain count to 5 (phase 3 needs its banks back).
        mmps = ctx.enter_context(tc.tile_pool(name="mmps", bufs=3, space="PSUM"))
        ph2ps_cm = tc.tile_pool(name="ph2ps", bufs=2, space="PSUM")
        ph2ps = ph2ps_cm.__enter__()
        _chain_i = 0

        def proj_ps_tile():
            nonlocal _chain_i
            _chain_i += 1
            pool = ph2ps if _chain_i % 2 == 0 else mmps
            return pool.tile([P, 512], F32, tag="mm512", name=f"pps{_chain_i}")

        for sc8 in range(NK):
            # K^T [d, s] for this 512-column block of s
            for dj in range(CT):
                ps = proj_ps_tile()
                for ci in range(CT):
                    nc.tensor.matmul(ps[:], wkt[:, ci, dj * P:(dj + 1) * P],
                                     xn[:, ci, sc8 * 512:(sc8 + 1) * 512],
                                     start=(ci == 0), stop=(ci == CT - 1))
                if dj % 3 == 0:
                    nc.scalar.activation(kt[:, dj, sc8 * 512:(sc8 + 1) * 512],
                                         ps[:], _AF.Identity,
                                         bias=bkp[:, dj:dj + 1])
                else:
                    nc.vector.tensor_scalar(
                        kt[:, dj, sc8 * 512:(sc8 + 1) * 512], ps[:],
                        bkp[:, dj:dj + 1], None, _ALU.add)
            # Q^T [d, t] (scaled by 1/sqrt(C); bqs prescaled on host)
            if sc8 < TCH:
                tch = sc8
                for dj in range(CT):
                    ps = proj_ps_tile()
                    for ci in range(CT):
                        nc.tensor.matmul(ps[:], wqt[:, ci, dj * P:(dj + 1) * P],
                                         xn[:, ci, tch * 512:(tch + 1) * 512],
                                         start=(ci == 0), stop=(ci == CT - 1))
                    if dj % 3 == 0:
                        nc.scalar.activation(qt[:, dj, tch * 512:(tch + 1) * 512],
                                             ps[:], _AF.Identity, scale=SCL,
                                             bias=bqs[:, dj:dj + 1])
                    else:
                        nc.vector.tensor_scalar(
                            qt[:, dj, tch * 512:(tch + 1) * 512], ps[:],
                            SCL, bqs[:, dj:dj + 1], _ALU.mult, _ALU.add)
            # V [s, d] for the four 128-token chunks in this block
            for sc in range(4 * sc8, 4 * sc8 + 4):
                ps = proj_ps_tile()
                for ci in range(CT):
                    nc.tensor.matmul(ps[:], xn[:, ci, sc * P:(sc + 1) * P],
                                     wvt[:, ci, :],
                                     start=(ci == 0), stop=(ci == CT - 1))
                nc.vector.tensor_add(out=vt[:, sc, :], in0=ps[:], in1=bvr[:])

        # ---- phase 3: attention, t-chunk at a time (phase-2 pool released)
        ph2ps_cm.__exit__(None, None, None)
        attnps = ctx.enter_context(tc.tile_pool(name="attnps", bufs=1, space="PSUM"))
        ph3ps = ctx.enter_context(tc.tile_pool(name="ph3ps", bufs=1, space="PSUM"))
        p3 = ctx.enter_context(tc.tile_pool(name="p3", bufs=3))
        p3b = ctx.enter_context(tc.tile_pool(name="p3b", bufs=2))

        for tch in range(TCH):
            t0 = tch * 512
            attn_ps = [attnps.tile([P, 512], F32, tag=f"attn{cj}",
                                   name=f"attn_ps{cj}") for cj in range(CT)]
            acc = p3b.tile([P, 512], F32, tag="acc")
            last_pch = []
            for sc in range(SCH):
                sps = mmps.tile([P, 512], F32, tag="mm512")
                for di in range(CT):
                    nc.tensor.matmul(sps[:], kt[:, di, sc * P:(sc + 1) * P],
                                     qt[:, di, t0:t0 + 512],
                                     start=(di == 0), stop=(di == CT - 1))
                pch = p3.tile([P, 512], BF16, tag="pch", name=f"pch_{sc}")
                nc.scalar.activation(pch[:], sps[:], _AF.Exp)
                # last two chunks go into the Z matmul directly so the DVE
                # accumulator is off the boundary critical path
                if sc == 0:
                    nc.vector.tensor_copy(out=acc[:], in_=pch[:])
                elif sc < SCH - 2:
                    nc.vector.tensor_add(out=acc[:], in0=acc[:], in1=pch[:])
                else:
                    last_pch.append(pch)
                for cj in range(CT):
                    nc.tensor.matmul(attn_ps[cj][:], vt[:, sc, cj * P:(cj + 1) * P],
                                     pch[:], start=(sc == 0), stop=(sc == SCH - 1))
            last_tc = (tch == TCH - 1)
            # Z replicated across partitions: ones^T @ acc + ones^T @ last P';
            # 1/Z via the ~51-ULP single-op approximate reciprocal (Z is
            # strictly positive, well inside its safe range)
            def z_chain():
                zrep_ps = ph3ps.tile([P, 512], F32, tag="zps", name="zrep_ps")
                nc.tensor.matmul(zrep_ps[:], onesq[:], acc[:], start=True,
                                 stop=False)
                for i, pch in enumerate(last_pch):
                    nc.tensor.matmul(zrep_ps[:], onesqb[:], pch[:], start=False,
                                     stop=(i == len(last_pch) - 1))
                zrep = p3b.tile([P, 512], F32, tag="zrep", name="zrep")
                nc.vector.reciprocal_approx_fast(out=zrep[:], in_=zrep_ps[:])
                return zrep

            if last_tc:
                # tail: normalize during the attn copy (outproj then needs no
                # per-element epilogue beyond the residual add)
                zrep = z_chain()
            # copy attn^T out of PSUM right away (frees the banks; for the
            # steady-state chunks 1/Z is applied after the wo matmul, which
            # it commutes with, so these copies wait on nothing).
            # All four on DVE, ahead of the reciprocal in its in-order queue,
            # so the outproj matmuls get their rhs with no detour.
            attn_sb = []
            for cj in range(CT):
                asb = p3.tile([P, 512], BF16, tag=f"asb{cj}", name=f"asb{cj}")
                if last_tc:
                    nc.vector.tensor_mul(out=asb[:], in0=attn_ps[cj][:],
                                         in1=zrep[:])
                else:
                    nc.vector.tensor_copy(out=asb[:], in_=attn_ps[cj][:])
                attn_sb.append(asb)
            if not last_tc:
                zrep = z_chain()
            # output projection on unnormalized attn; epilogue applies 1/Z
            for dj in range(CT):
                ops = mmps.tile([P, 512], F32, tag="mm512")
                for cj in range(CT):
                    nc.tensor.matmul(ops[:], wot[:, cj, dj * P:(dj + 1) * P],
                                     attn_sb[cj][:],
                                     start=(cj == 0), stop=(cj == CT - 1))
                rt = p3.tile([P, 512], BF16, tag="rt")
                nc.gpsimd.dma_start(rt[:], x_ext[dj * P:(dj + 1) * P, t0:t0 + 512])
                rb = p3.tile([P, 512], F32, tag="rb")
                nc.scalar.activation(rb[:], rt[:], _AF.Identity,
                                     bias=bop[:, dj:dj + 1])
                osb = p3.tile([P, 512], F32, tag="osb")
                if last_tc:
                    nc.vector.tensor_add(out=osb[:], in0=ops[:], in1=rb[:])
                else:
                    nc.vector.tensor_mul(out=osb[:], in0=ops[:], in1=zrep[:])
                    nc.vector.tensor_add(out=osb[:], in0=osb[:], in1=rb[:])
                out_eng = nc.sync if dj % 2 == 0 else nc.scalar
                out_eng.dma_start(out_ext[dj * P:(dj + 1) * P, t0:t0 + 512], osb[:])

    nc.compile()
    return nc


_graph_cache = None


def _get_graph():
    global _graph_cache
    if _graph_cache is None:
        _graph_cache = _build_graph()
    return _graph_cache


def _prep_constants(gn_scale, gn_bias, wq, bq, wk, bk, wv, bv, wo, bo):
    def p_layout(v):  # [C] -> [P, CT] with channel c = ci*P + p
        return np.ascontiguousarray(v.reshape(CT, P).T.astype(np.float32))

    def w_t_layout(w):  # [d_out, c_in] -> wT [c, d] -> [P, CT, C] bf16
        wt = w.T.astype(np.float32)  # [c, d]
        return np.ascontiguousarray(
            wt.reshape(CT, P, C).transpose(1, 0, 2)).astype(ml_dtypes.bfloat16)

    gind = np.zeros((P, CT, NUM_GROUPS), np.float32)
    gindt = np.zeros((NUM_GROUPS, CT, P), np.float32)
    for ti in range(CT):
        for p in range(P):
            g = (ti * P + p) // GSIZE
            gind[p, ti, g] = 1.0
            gindt[g, ti, p] = 1.0

    return {
        "wqt": w_t_layout(wq), "wkt": w_t_layout(wk),
        "wvt": w_t_layout(wv), "wot": w_t_layout(wo),
        "bqs": p_layout(bq * SCL), "bkp": p_layout(bk), "bop": p_layout(bo),
        "bvrep": np.ascontiguousarray(
            np.broadcast_to(bv.astype(np.float32), (P, C))),
        "gnsc": p_layout(gn_scale), "gnbi": p_layout(gn_bias),
        "gind": gind, "gindt": gindt,
        "ones_sq": np.ones((P, P), np.float32),
        "ones_sq_bf": np.ones((P, P), ml_dtypes.bfloat16),
    }


def kernel(x, gn_scale, gn_bias, wq, bq, wk, bk, wv, bv, wo, bo):
    global last_exec_time_ns, last_results
    x = np.asarray(x, dtype=np.float32)
    consts = _prep_constants(
        np.asarray(gn_scale, np.float32), np.asarray(gn_bias, np.float32),
        np.asarray(wq, np.float32), np.asarray(bq, np.float32),
        np.asarray(wk, np.float32), np.asarray(bk, np.float32),
        np.asarray(wv, np.float32), np.asarray(bv, np.float32),
        np.asarray(wo, np.float32), np.asarray(bo, np.float32))

    in_maps = []
    for core in range(N_CORES):
        b, h = core // 2, core % 2
        x2d = x[b].reshape(C, N)
        # rotate tokens so this core's query half is first; ship as bf16
        xp = np.ascontiguousarray(
            np.concatenate([x2d[:, h * T:(h + 1) * T],
                            x2d[:, (1 - h) * T:(2 - h) * T]],
                           axis=1)).astype(ml_dtypes.bfloat16)
        m = {"x": xp}
        m.update(consts)
        in_maps.append(m)

    nc = _get_graph()
    trace = bool(int(os.environ.get("BASS_KERNEL_TRACE", "0")))
    res = run_bass_kernel_spmd(nc, in_maps, core_ids=list(range(N_CORES)),
                               trace=trace)
    last_exec_time_ns = res.exec_time_ns
    last_results = res

    out = np.empty((B, C, N), np.float32)
    for core in range(N_CORES):
        b, h = core // 2, core % 2
        out[b][:, h * T:(h + 1) * T] = res.results[core]["out"]
    return out.reshape(B, C, H, W)



# revision 3
# speedup vs baseline: 1.8516x; 1.8516x over previous
"""Trainium2 Bass kernel: GroupNorm + single-head self-attention + residual.

Reference computation (B=4, C=512, H=W=64, N=4096 tokens):
    h  = GroupNorm32(x) ; hf = h tokens x channels
    q/k/v = hf @ W{q,k,v}^T + b
    attn  = softmax(q k^T / sqrt(C)) @ v
    out   = attn @ Wo^T + bo  (+ x residual)

Sharding: 8 cores, core c -> batch b=c//2, query-half h=c%2 (2048 queries).
Each core receives x[b] with tokens rotated so its query half is first; the
SPMD graph is identical on every core. K/V are computed for all 4096 tokens
on both cores of a pair (cheaper than a collective at this size).

All heavy matmuls run in fp8e4 (e4m3, max 240) with perf_mode=DoubleRow:
the PE packs two fp8 weights per cell, so each MM contracts 256 (two
128-chunks addressed via a 3D AP [128, 2, free]) and replaces two bf16
MMs.  Accuracy budget: the residual dominates the output norm (the
attention term is ~2.6% of it), so attention-path quantization error is
suppressed ~40x; numpy simulation of this exact scheme gives rel err
5.6e-3 vs the 2e-2 gate.

Scaling scheme (fp8 wants ~unit-sigma values):
    weights shipped as 16*W^T fp8 (sigma ~0.7)
    xn (normalized x) fp8 sigma 1;  qt = ps/16 + bq (sigma 1)
    kt = ps/16 + bk (sigma 1);      vt = ps = 16*(v-bv) (sigma 16)
    scores = qt.kt raw; pch = exp(SCL*s - 2) fp8 (max ~49 < 240)
    Z via ones-matmul on fp32/fp8 accumulators; zrep = 1/Z
    asb = attn_ps * zrep = 16*attn fp8 (sigma 0.42)
    ops = asb @ 16wo = 256*out;  osb = ops/256 + (xres + bo')  bf16
    bo' = bo + Wo@bv (host-folded; softmax rows sum to 1 so +bv passes
    through attention exactly)

On-chip layouts (partition dim first):
    x8  [128, 4, 4096] fp8    raw x, channel c = ci*128+p
    xn  [128, 4, 4096] fp8    normalized
    kt  [128, 4, 4096] fp8    K^T, d on partitions
    qt  [128, 4, 2048] fp8    Q^T, d on partitions
    vt  [128, 32, 512] fp8    16*V, tokens on partitions
GroupNorm stats via DVE bn_stats/bn_aggr (one pass, frees ACT for the
prologue); group reduction via indicator matmuls as before.
"""

import math
import os

import numpy as np
import ml_dtypes

import concourse.bass as bass
import concourse.bacc as bacc
import concourse.mybir as mybir
import concourse.tile as tile
from concourse.bass_utils import run_bass_kernel_spmd

# ----------------------------------------------------------------------------
# Problem constants (hardcoded per spec: x [4, 512, 64, 64] f32)
B, C, H, W = 4, 512, 64, 64
N = H * W          # 4096 tokens
T = N // 2         # 2048 queries per core
P = 128
CT = C // P        # 4 channel tiles
NUM_GROUPS = 32
GSIZE = C // NUM_GROUPS  # 16 channels per group
EPS = 1e-5
SCL = 1.0 / math.sqrt(C)
ESHIFT = 2.0       # softmax exp shift: pch = exp(SCL*s - ESHIFT)
WS = 16.0          # weight prescale for fp8
N_CORES = 8
F32 = mybir.dt.float32
BF16 = mybir.dt.bfloat16
FP8 = mybir.dt.float8e4

_AF = mybir.ActivationFunctionType
_ALU = mybir.AluOpType
_DR = mybir.MatmulPerfMode.DoubleRow

# set by kernel() when BASS_KERNEL_TRACE=1 (used by test.py)
last_exec_time_ns = None
last_results = None


def _build_graph():
    from contextlib import ExitStack

    # Bacc (not plain Bass): its compile() runs generate_event_semaphores,
    # which splits multi-wait sync_info into InstEventSemaphores — this
    # walrus build rejects >2 waits per instruction.
    nc = bacc.Bacc("TRN2", target_bir_lowering=False)

    x_ext = nc.declare_dram_parameter("x8", [C, N], FP8, isOutput=False)
    xres_ext = nc.declare_dram_parameter("xres", [C, T], BF16, isOutput=False)
    wqt_ext = nc.declare_dram_parameter("wqt", [P, CT, C], FP8, isOutput=False)
    wkt_ext = nc.declare_dram_parameter("wkt", [P, CT, C], FP8, isOutput=False)
    wvt_ext = nc.declare_dram_parameter("wvt", [P, CT, C], FP8, isOutput=False)
    wot_ext = nc.declare_dram_parameter("wot", [P, CT, C], FP8, isOutput=False)
    bqs_ext = nc.declare_dram_parameter("bqs", [P, CT], F32, isOutput=False)
    bkp_ext = nc.declare_dram_parameter("bkp", [P, CT], F32, isOutput=False)
    bop_ext = nc.declare_dram_parameter("bop", [P, CT], F32, isOutput=False)
    gsc_ext = nc.declare_dram_parameter("gnsc", [P, CT], F32, isOutput=False)
    gbi_ext = nc.declare_dram_parameter("gnbi", [P, CT], F32, isOutput=False)
    gind_ext = nc.declare_dram_parameter("gind", [P, CT, NUM_GROUPS], F32, isOutput=False)
    gindt_ext = nc.declare_dram_parameter("gindt", [NUM_GROUPS, CT, P], F32, isOutput=False)
    onesq_ext = nc.declare_dram_parameter("ones_sq", [P, P], F32, isOutput=False)
    ones8_ext = nc.declare_dram_parameter("ones_f8", [P, 2, P], FP8, isOutput=False)
    out_ext = nc.declare_dram_parameter("out", [C, T], BF16, isOutput=True)

    SCH = N // P     # 32 s-chunks of 128
    SCP = SCH // 2   # 16 s-chunk pairs
    NK = N // 512    # 8 s-blocks of 512
    TCH = T // 512   # 4 t-chunks of 512

    with tile.TileContext(nc) as tc, ExitStack() as ctx:
        consts = ctx.enter_context(tc.tile_pool(name="consts", bufs=1))
        big = ctx.enter_context(tc.tile_pool(name="big", bufs=1))
        small = ctx.enter_context(tc.tile_pool(name="small", bufs=1))

        # ---- constants into SBUF (gpsimd HWDGE queue; x8 owns sync+scalar
        # +vector so its 2MB lands as fast as possible)
        wqt = consts.tile([P, CT, C], FP8, tag="wqt")
        wkt = consts.tile([P, CT, C], FP8, tag="wkt")
        wvt = consts.tile([P, CT, C], FP8, tag="wvt")
        wot = consts.tile([P, CT, C], FP8, tag="wot")
        bqs = consts.tile([P, CT], F32, tag="bqs")
        bkp = consts.tile([P, CT], F32, tag="bkp")
        bop = consts.tile([P, CT], F32, tag="bop")
        gsc = consts.tile([P, CT], F32, tag="gsc")
        gbi = consts.tile([P, CT], F32, tag="gbi")
        gind = consts.tile([P, CT, NUM_GROUPS], F32, tag="gind")
        gindt = consts.tile([NUM_GROUPS, CT, P], F32, tag="gindt")
        onesq = consts.tile([P, P], F32, tag="onesq")
        ones8 = consts.tile([P, 2, P], FP8, tag="ones8")
        negc = consts.tile([P, 1], F32, tag="negc")
        nc.vector.memset(negc[:], -ESHIFT)

        # ---- persistent big tensors
        x8 = big.tile([P, CT, N], FP8, tag="x8")
        xn = big.tile([P, CT, N], FP8, tag="xn")
        kt = big.tile([P, CT, N], FP8, tag="kt")
        vt = big.tile([P, SCH, C], FP8, tag="vt")
        qt = big.tile([P, CT, T], FP8, tag="qt")
        xres = big.tile([P, CT, T], BF16, tag="xres")

        # ---- x loads first (x is the critical path): each tile split into
        # column thirds-ish across three HWDGE queues so tile ti completes
        # early and stats chase the DMA
        Nh = N // 2
        for ti in range(CT):
            nc.sync.dma_start(x8[:, ti, 0:Nh], x_ext[ti * P:(ti + 1) * P, 0:Nh])
            nc.scalar.dma_start(x8[:, ti, Nh:N], x_ext[ti * P:(ti + 1) * P, Nh:N])
        # small consts then weights on gpsimd queue; xres (needed only in
        # phase 4) last
        nc.gpsimd.dma_start(gind[:], gind_ext[:])
        nc.gpsimd.dma_start(gindt[:], gindt_ext[:])
        nc.gpsimd.dma_start(gsc[:], gsc_ext[:])
        nc.gpsimd.dma_start(gbi[:], gbi_ext[:])
        nc.gpsimd.dma_start(bqs[:], bqs_ext[:])
        nc.gpsimd.dma_start(bkp[:], bkp_ext[:])
        nc.gpsimd.dma_start(bop[:], bop_ext[:])
        nc.gpsimd.dma_start(onesq[:], onesq_ext[:])
        nc.gpsimd.dma_start(ones8[:], ones8_ext[:])
        nc.gpsimd.dma_start(wkt[:], wkt_ext[:])
        nc.gpsimd.dma_start(wqt[:], wqt_ext[:])
        nc.gpsimd.dma_start(wvt[:], wvt_ext[:])
        nc.gpsimd.dma_start(wot[:], wot_ext[:])
        for ti in range(CT):
            nc.gpsimd.dma_start(xres[:, ti, :], xres_ext[ti * P:(ti + 1) * P, :])

        # ---- phase 1: GroupNorm stats via bn_stats (one DVE pass computes
        # count/mean/M2 per 512-col chunk), bn_aggr per tile, then group
        # reduction via indicator matmuls.  Runs per column half so each
        # starts as soon as that half's DMA lands.
        bns = small.tile([P, CT, 8, 6], F32, tag="bns")
        cv = small.tile([P, CT, 2], F32, tag="cv")
        A_sb = small.tile([P, CT], F32, tag="A_sb")
        B_sb = small.tile([P, CT], F32, tag="B_sb")
        with (
            tc.tile_pool(name="ph1ps", bufs=2, space="PSUM") as ph1ps,
            tc.tile_pool(name="ph1sb", bufs=2) as ph1sb,
        ):
            for hf in range(2):
                for ti in range(CT):
                    for cc in range(4):
                        col = hf * 4 + cc
                        nc.vector.bn_stats(
                            bns[:, ti, col, :],
                            x8[:, ti, col * 512:(col + 1) * 512])
            gmin = ph1sb.tile([P, CT, 2], F32, tag="gmin")
            mean2 = ph1sb.tile([P, CT], F32, tag="mean2")
            for ti in range(CT):
                nc.vector.bn_aggr(cv[:, ti, :], bns[:, ti, :, :])
            # per-channel (mean, E[x^2]) for the group matmul
            nc.vector.tensor_mul(out=mean2[:], in0=cv[:, :, 0], in1=cv[:, :, 0])
            nc.vector.tensor_copy(out=gmin[:, :, 0], in_=cv[:, :, 0])
            nc.vector.tensor_tensor(gmin[:, :, 1], cv[:, :, 1], mean2[:],
                                    _ALU.add)
            # group sums: gind holds 1/GSIZE so this directly averages the
            # 16 channels of each group -> [32, (mean_g, E[x^2]_g)]
            gs_ps = ph1ps.tile([NUM_GROUPS, 2], F32, tag="gsps")
            for ti in range(CT):
                nc.tensor.matmul(gs_ps[:], gind[:, ti, :], gmin[:, ti, :],
                                 start=(ti == 0), stop=(ti == CT - 1))
            gstats = ph1sb.tile([NUM_GROUPS, 2], F32, tag="gstats")
            nc.vector.tensor_copy(out=gstats[:], in_=gs_ps[:])
            m2 = ph1sb.tile([NUM_GROUPS, 1], F32, tag="m2")
            nc.vector.tensor_mul(out=m2[:], in0=gstats[:, 0:1], in1=gstats[:, 0:1])
            var = ph1sb.tile([NUM_GROUPS, 1], F32, tag="var")
            nc.vector.tensor_tensor(var[:], gstats[:, 1:2], m2[:], _ALU.subtract)
            eps_t = ph1sb.tile([NUM_GROUPS, 1], F32, tag="eps")
            nc.vector.memset(eps_t[:], EPS)
            std = ph1sb.tile([NUM_GROUPS, 1], F32, tag="std")
            nc.scalar.activation(std[:], var[:], _AF.Sqrt, bias=eps_t[:])
            gmr = ph1sb.tile([NUM_GROUPS, 2], F32, tag="gmr")
            nc.vector.tensor_copy(out=gmr[:, 0:1], in_=gstats[:, 0:1])
            nc.vector.reciprocal(gmr[:, 1:2], std[:])

            # broadcast (mean, rstd) back to channel layout; A/B affine.
            chan_all = ph1sb.tile([P, CT, 2], F32, tag="chanall")
            for ti in range(CT):
                chan_ps = ph1ps.tile([P, 2], F32, tag="chanps")
                nc.tensor.matmul(chan_ps[:], gindt[:, ti, :], gmr[:],
                                 start=True, stop=True)
                nc.vector.tensor_copy(out=chan_all[:, ti, :], in_=chan_ps[:])
            nc.vector.tensor_mul(out=A_sb[:], in0=chan_all[:, :, 1], in1=gsc[:])
            tmpm = ph1sb.tile([P, CT], F32, tag="tmpm")
            nc.vector.tensor_mul(out=tmpm[:], in0=chan_all[:, :, 0], in1=A_sb[:])
            nc.vector.tensor_tensor(B_sb[:], gbi[:], tmpm[:], _ALU.subtract)

            # normalize x8 -> xn (fp8), column-chunk first so projections can
            # start on chunk 0 while later chunks normalize; DVE is faster
            # than ACT here so it takes 2 of every 3 slices
            for cc in range(N // 512):
                for ti in range(CT):
                    src = x8[:, ti, cc * 512:(cc + 1) * 512]
                    dst = xn[:, ti, cc * 512:(cc + 1) * 512]
                    if (cc * CT + ti) % 3 == 0:
                        nc.scalar.activation(dst, src, _AF.Identity,
                                             scale=A_sb[:, ti:ti + 1],
                                             bias=B_sb[:, ti:ti + 1])
                    else:
                        nc.vector.tensor_scalar(dst, src,
                                                A_sb[:, ti:ti + 1],
                                                B_sb[:, ti:ti + 1],
                                                _ALU.mult, _ALU.add)

        # ---- phase 2: projections (fp8 DoubleRow: 2 MMs per 512-contraction
        # chain), ordered by xn column chunk so they chase the normalize pass
        mmps = ctx.enter_context(tc.tile_pool(name="mmps", bufs=3, space="PSUM"))
        ph2ps_cm = tc.tile_pool(name="ph2ps", bufs=2, space="PSUM")
        ph2ps = ph2ps_cm.__enter__()
        _chain_i = 0

        def proj_ps_tile():
            nonlocal _chain_i
            _chain_i += 1
            pool = ph2ps if _chain_i % 2 == 0 else mmps
            return pool.tile([P, 512], F32, tag="mm512", name=f"pps{_chain_i}")

        def dr_chain(ps, lhs_fn, rhs_fn):
            # two DoubleRow MMs contracting ci-pairs (0,1) then (2,3)
            for half in range(2):
                nc.tensor.matmul(ps[:], lhs_fn(2 * half), rhs_fn(2 * half),
                                 start=(half == 0), stop=(half == 1),
                                 perf_mode=_DR)

        for sc8 in range(NK):
            s0 = sc8 * 512
            # K^T [d, s] for this 512-column block of s
            for dj in range(CT):
                ps = proj_ps_tile()
                dr_chain(ps,
                         lambda ci, dj=dj: wkt[:, ci:ci + 2, dj * P:(dj + 1) * P],
                         lambda ci, s0=s0: xn[:, ci:ci + 2, s0:s0 + 512])
                if dj % 2 == 0:
                    nc.scalar.activation(kt[:, dj, s0:s0 + 512], ps[:],
                                         _AF.Identity, scale=1.0 / WS,
                                         bias=bkp[:, dj:dj + 1])
                else:
                    nc.vector.tensor_scalar(
                        kt[:, dj, s0:s0 + 512], ps[:],
                        1.0 / WS, bkp[:, dj:dj + 1], _ALU.mult, _ALU.add)
            # Q^T [d, t]
            if sc8 < TCH:
                for dj in range(CT):
                    ps = proj_ps_tile()
                    dr_chain(ps,
                             lambda ci, dj=dj: wqt[:, ci:ci + 2, dj * P:(dj + 1) * P],
                             lambda ci, s0=s0: xn[:, ci:ci + 2, s0:s0 + 512])
                    if dj % 2 == 0:
                        nc.scalar.activation(qt[:, dj, s0:s0 + 512], ps[:],
                                             _AF.Identity, scale=1.0 / WS,
                                             bias=bqs[:, dj:dj + 1])
                    else:
                        nc.vector.tensor_scalar(
                            qt[:, dj, s0:s0 + 512], ps[:],
                            1.0 / WS, bqs[:, dj:dj + 1], _ALU.mult, _ALU.add)
            # V [s, d] (as 16*v) for the four 128-token chunks in this block
            for sc in range(4 * sc8, 4 * sc8 + 4):
                ps = proj_ps_tile()
                dr_chain(ps,
                         lambda ci, sc=sc: xn[:, ci:ci + 2, sc * P:(sc + 1) * P],
                         lambda ci: wvt[:, ci:ci + 2, :])
                if sc % 2 == 0:
                    nc.vector.tensor_copy(out=vt[:, sc, :], in_=ps[:])
                else:
                    nc.scalar.activation(vt[:, sc, :], ps[:], _AF.Identity)

        # ---- phase 3: attention, t-chunk at a time (phase-2 pool released)
        ph2ps_cm.__exit__(None, None, None)
        attnps = ctx.enter_context(tc.tile_pool(name="attnps", bufs=1, space="PSUM"))
        ph3ps = ctx.enter_context(tc.tile_pool(name="ph3ps", bufs=1, space="PSUM"))
        p3 = ctx.enter_context(tc.tile_pool(name="p3", bufs=3))
        p3b = ctx.enter_context(tc.tile_pool(name="p3b", bufs=2))

        for tch in range(TCH):
            t0 = tch * 512
            attn_ps = [attnps.tile([P, 512], F32, tag=f"attn{cj}",
                                   name=f"attn_ps{cj}") for cj in range(CT)]
            # two Z accumulators so the adds run on two engines in parallel
            acc0 = p3b.tile([P, 512], F32, tag="acc0")
            acc1 = p3b.tile([P, 512], F32, tag="acc1")
            last_pch = None
            for scp in range(SCP):
                pch = p3.tile([P, 2, 512], FP8, tag="pch", name=f"pch_{scp}")
                for j in range(2):
                    sc = 2 * scp + j
                    sps = mmps.tile([P, 512], F32, tag="mm512")
                    for half in range(2):
                        di = 2 * half
                        nc.tensor.matmul(
                            sps[:], kt[:, di:di + 2, sc * P:(sc + 1) * P],
                            qt[:, di:di + 2, t0:t0 + 512],
                            start=(half == 0), stop=(half == 1), perf_mode=_DR)
                    nc.scalar.activation(pch[:, j, :], sps[:], _AF.Exp,
                                         scale=SCL, bias=negc[:])
                # last pair goes into the Z matmul directly so the
                # accumulators are off the boundary critical path
                if scp == 0:
                    nc.gpsimd.tensor_copy(out=acc0[:], in_=pch[:, 0, :])
                    nc.vector.tensor_copy(out=acc1[:], in_=pch[:, 1, :])
                elif scp < SCP - 1:
                    nc.gpsimd.tensor_add(out=acc0[:], in0=acc0[:], in1=pch[:, 0, :])
                    nc.vector.tensor_add(out=acc1[:], in0=acc1[:], in1=pch[:, 1, :])
                else:
                    last_pch = pch
                for cj in range(CT):
                    nc.tensor.matmul(
                        attn_ps[cj][:],
                        vt[:, 2 * scp:2 * scp + 2, cj * P:(cj + 1) * P],
                        pch[:, :, :],
                        start=(scp == 0), stop=(scp == SCP - 1), perf_mode=_DR)
            # Z replicated across partitions: ones^T @ (acc0 + acc1 + last
            # pair); 1/Z via the ~51-ULP approximate reciprocal (Z > 0)
            zrep_ps = ph3ps.tile([P, 512], F32, tag="zps", name="zrep_ps")
            nc.tensor.matmul(zrep_ps[:], onesq[:], acc0[:], start=True,
                             stop=False)
            nc.tensor.matmul(zrep_ps[:], onesq[:], acc1[:], start=False,
                             stop=False)
            nc.tensor.matmul(zrep_ps[:], ones8[:], last_pch[:, :, :],
                             start=False, stop=True, perf_mode=_DR)
            zrep = p3b.tile([P, 512], F32, tag="zrep", name="zrep")
            nc.vector.reciprocal_approx_fast(out=zrep[:], in_=zrep_ps[:])

            # asb = 16*attn fp8 (normalized; unnormalized would clip fp8)
            asb = p3.tile([P, CT, 512], FP8, tag="asb")
            for cj in range(CT):
                nc.vector.tensor_mul(out=asb[:, cj, :], in0=attn_ps[cj][:],
                                     in1=zrep[:])
            # output projection (DoubleRow over cj pairs) + epilogue:
            # osb = ops/256 + (xres + bo')
            for dj in range(CT):
                ops = mmps.tile([P, 512], F32, tag="mm512")
                for half in range(2):
                    cj = 2 * half
                    nc.tensor.matmul(ops[:], wot[:, cj:cj + 2, dj * P:(dj + 1) * P],
                                     asb[:, cj:cj + 2, :],
                                     start=(half == 0), stop=(half == 1),
                                     perf_mode=_DR)
                rb = p3.tile([P, 512], F32, tag="rb")
                nc.scalar.activation(rb[:], xres[:, dj, t0:t0 + 512],
                                     _AF.Identity, bias=bop[:, dj:dj + 1])
                osb = p3.tile([P, 512], BF16, tag="osb")
                nc.vector.scalar_tensor_tensor(
                    osb[:], ops[:], 1.0 / (WS * WS), rb[:],
                    _ALU.mult, _ALU.add)
                out_eng = nc.sync if dj % 2 == 0 else nc.scalar
                out_eng.dma_start(out_ext[dj * P:(dj + 1) * P, t0:t0 + 512], osb[:])

    nc.compile()
    return nc


_graph_cache = None


def _get_graph():
    global _graph_cache
    if _graph_cache is None:
        _graph_cache = _build_graph()
    return _graph_cache


def _prep_constants(gn_scale, gn_bias, wq, bq, wk, bk, wv, bv, wo, bo):
    def p_layout(v):  # [C] -> [P, CT] with channel c = ci*P + p
        return np.ascontiguousarray(v.reshape(CT, P).T.astype(np.float32))

    def w_t_layout(w):  # [d_out, c_in] -> 16*wT [c, d] -> [P, CT, C] fp8
        wt = (WS * w.T).astype(np.float32)  # [c, d]
        return np.ascontiguousarray(
            wt.reshape(CT, P, C).transpose(1, 0, 2)).astype(
                ml_dtypes.float8_e4m3)

    gind = np.zeros((P, CT, NUM_GROUPS), np.float32)
    gindt = np.zeros((NUM_GROUPS, CT, P), np.float32)
    for ti in range(CT):
        for p in range(P):
            g = (ti * P + p) // GSIZE
            gind[p, ti, g] = 1.0 / GSIZE
            gindt[g, ti, p] = 1.0

    bo_fold = bo + wo @ bv  # bv passes through softmax: fold into out bias

    return {
        "wqt": w_t_layout(wq), "wkt": w_t_layout(wk),
        "wvt": w_t_layout(wv), "wot": w_t_layout(wo),
        "bqs": p_layout(bq), "bkp": p_layout(bk), "bop": p_layout(bo_fold),
        "gnsc": p_layout(gn_scale), "gnbi": p_layout(gn_bias),
        "gind": gind, "gindt": gindt,
        "ones_sq": np.ones((P, P), np.float32),
        "ones_f8": np.ones((P, 2, P), ml_dtypes.float8_e4m3),
    }


def kernel(x, gn_scale, gn_bias, wq, bq, wk, bk, wv, bv, wo, bo):
    global last_exec_time_ns, last_results
    x = np.asarray(x, dtype=np.float32)
    consts = _prep_constants(
        np.asarray(gn_scale, np.float32), np.asarray(gn_bias, np.float32),
        np.asarray(wq, np.float32), np.asarray(bq, np.float32),
        np.asarray(wk, np.float32), np.asarray(bk, np.float32),
        np.asarray(wv, np.float32), np.asarray(bv, np.float32),
        np.asarray(wo, np.float32), np.asarray(bo, np.float32))

    in_maps = []
    for core in range(N_CORES):
        b, h = core // 2, core % 2
        x2d = x[b].reshape(C, N)
        # rotate tokens so this core's query half is first
        xrot = np.ascontiguousarray(
            np.concatenate([x2d[:, h * T:(h + 1) * T],
                            x2d[:, (1 - h) * T:(2 - h) * T]],
                           axis=1))
        m = {"x8": xrot.astype(ml_dtypes.float8_e4m3),
             "xres": np.ascontiguousarray(xrot[:, :T]).astype(
                 ml_dtypes.bfloat16)}
        m.update(consts)
        in_maps.append(m)

    nc = _get_graph()
    trace = bool(int(os.environ.get("BASS_KERNEL_TRACE", "0")))
    res = run_bass_kernel_spmd(nc, in_maps, core_ids=list(range(N_CORES)),
                               trace=trace)
    last_exec_time_ns = res.exec_time_ns
    last_results = res

    out = np.empty((B, C, N), np.float32)
    for core in range(N_CORES):
        b, h = core // 2, core % 2
        out[b][:, h * T:(h + 1) * T] = res.results[core]["out"].astype(
            np.float32)
    return out.reshape(B, C, H, W)
